# revision 1
# baseline (speedup 1.0000x reference)
"""Multi-head attention TRN2 kernel v2 (8-core SPMD, batch x head-half).

Per core (c): batch b = c % 4, head-half g = c // 4 (8 of 16 heads).
Two passes of 4 heads (2 pairs pp each). bf16 matmul dataflow:
  QT/KT [2x64 hd, tok] pair-major; V token-major with ones column per head
  (softmax denominator from the attn@V matmul, row 64 of the accumulator).

v3: software-pipelined (ACT exp runs continuously; PE fills slack with
next-pass projections and prev-pass output projection), and every K=128
contraction matmul is row-split into a concurrent pair of K=64 PE tiles
at base partitions 0/64 (HW-measured 280ns/pair vs 400ns for a single
full-K matmul whose LDWEIGHTS cannot be hidden). V projection for both
passes is merged (N=512 moving). Bulk DMA goes on the Pool queue.
PSUM: 3 rotating [128,1024] slots + 1 oAB accumulator = 8 banks.
"""
import numpy as np
import concourse.bacc as bacc
import concourse.mybir as mybir
import concourse.tile as tile
from concourse import bass_utils

BF16 = mybir.dt.bfloat16
F32 = mybir.dt.float32
AF = mybir.ActivationFunctionType

S, D = 2048, 1024
CH = 512          # token chunk for stage A
NCH = S // CH     # 4
NKT = S // 128    # 16 key-token tiles
NQ = S // 512     # 4 query blocks of 512


def build(reps: int = 1, phases: str = "abc"):
    nc = bacc.Bacc("TRN2", target_bir_lowering=False, debug=False, num_devices=8)
    xq_d = nc.dram_tensor("xq", [D, S], BF16, kind="ExternalInput")
    xk_d = nc.dram_tensor("xk", [D, S], BF16, kind="ExternalInput")
    xv_d = nc.dram_tensor("xv", [D, S], BF16, kind="ExternalInput")
    wq_d = nc.dram_tensor("wq", [D, 512], BF16, kind="ExternalInput")
    wk_d = nc.dram_tensor("wk", [D, 512], BF16, kind="ExternalInput")
    wv_d = nc.dram_tensor("wv", [D, 512], BF16, kind="ExternalInput")
    wo_d = nc.dram_tensor("wo", [512, D], BF16, kind="ExternalInput")
    bq_d = nc.dram_tensor("bq", [4, 128, 1], F32, kind="ExternalInput")
    bk_d = nc.dram_tensor("bk", [4, 128, 1], F32, kind="ExternalInput")
    bv_d = nc.dram_tensor("bv", [1, 512], F32, kind="ExternalInput")
    out_d = [
        nc.dram_tensor(f"out{p}", [S, D], F32, kind="ExternalOutput") for p in (0, 1)
    ]

    with tile.TileContext(nc) as tc:
        with (
            tc.tile_pool(name="pers", bufs=1) as pers,
            tc.tile_pool(name="xkp", bufs=4) as xkp,
            tc.tile_pool(name="xqp", bufs=2) as xqp,
            tc.tile_pool(name="xvp", bufs=2) as xvp,
            tc.tile_pool(name="wp", bufs=2) as wp,
            tc.tile_pool(name="qkvp", bufs=2) as qkvp,
            tc.tile_pool(name="ptp", bufs=10) as ptp,
            tc.tile_pool(name="atp", bufs=2) as atp,
            tc.tile_pool(name="smp", bufs=2) as smp,
            tc.tile_pool(name="rbp", bufs=3) as rbp,
            tc.tile_pool(name="ocp", bufs=2) as ocp,
            tc.tile_pool(name="stp", bufs=3, space="PSUM") as stp,
            tc.tile_pool(name="op", bufs=1, space="PSUM") as op,
            tc.tile_pool(name="drp", bufs=3, space="DRAM") as drp,
        ):
            bq_sb = pers.tile([128, 4], F32, tag="bq")
            bk_sb = pers.tile([128, 4], F32, tag="bk")
            for m in range(4):
                nc.sync.dma_start(bq_sb[:, m : m + 1], bq_d[m])
                nc.sync.dma_start(bk_sb[:, m : m + 1], bk_d[m])
            bv_sb = pers.tile([128, 512], F32, tag="bv")
            nc.sync.dma_start(bv_sb[:], bv_d[:].to_broadcast((128, 512)))
            # dummy exp: pulls the one-time ACT Exp-table load into the
            # prologue so the first real softmax tile isn't delayed by it
            warm = pers.tile([128, 4], BF16, tag="warm")
            nc.scalar.activation(warm[:], bq_sb[:], AF.Exp, scale=0.0)

            def pass_tiles(ps):
                QT = [
                    qkvp.tile([128, S], BF16, tag=f"qt{pp}", name=f"QT{ps}{pp}")
                    for pp in range(2)
                ]
                KT = [
                    qkvp.tile([128, S], BF16, tag=f"kt{pp}", name=f"KT{ps}{pp}")
                    for pp in range(2)
                ]
                v_sb = qkvp.tile([128, NKT * 264], BF16, tag="v", name=f"v{ps}")
                AT = [
                    atp.tile([128, S], BF16, tag=f"at{pp}", name=f"AT{ps}{pp}")
                    for pp in range(2)
                ]
                return dict(QT=QT, KT=KT, v=v_sb, AT=AT)

            def load_pass_weights(ps, T):
                cs = slice(ps * 256, (ps + 1) * 256)
                T["wq2"] = wp.tile([128, 2048], BF16, tag="wq2", name=f"wq2{ps}")
                T["wk2"] = wp.tile([128, 2048], BF16, tag="wk2", name=f"wk2{ps}")
                for w_sb, w_d in ((T["wq2"], wq_d), (T["wk2"], wk_d)):
                    nc.gpsimd.dma_start(
                        w_sb[:].rearrange("p (k m) -> p k m", k=8),
                        w_d[:, cs].rearrange("(k p) m -> p k m", p=128),
                    )

            def load_wv_full():
                wv = wp.tile([128, 4096], BF16, tag="wv", bufs=1, name="wvfull")
                nc.gpsimd.dma_start(
                    wv[:].rearrange("p (k m) -> p k m", k=8),
                    wv_d[:, :].rearrange("(k p) m -> p k m", p=128),
                )
                return wv

            def memset_ones(T):
                for t in range(NKT):
                    vv = T["v"][:, t * 264 : (t + 1) * 264].rearrange(
                        "p (h c) -> p h c", h=4
                    )
                    nc.vector.memset(vv[:, :, 64:66], 0.0)
                    nc.vector.memset(vv[:, :, 64:65], 1.0)

            def a_k_chunk(ps, ch, T, pps=(0, 1)):
                """K projection for chunk ch of pass ps (pairs in pps)."""
                toks = slice(ch * CH, (ch + 1) * CH)
                xk_ch = xkp.tile([128, 8 * CH], BF16, tag="xk",
                                 name=f"xk{ps}{ch}{pps[0]}")
                nc.gpsimd.dma_start(
                    xk_ch[:].rearrange("p (k m) -> p k m", k=8),
                    xk_d[:, toks].rearrange("(k p) m -> p k m", p=128),
                )
                for pp in pps:
                    m = 2 * ps + pp
                    pt = stp.tile([128, 1024], F32, tag="st", name=f"ak{ps}{ch}{pp}")
                    for k in range(8):
                        lsl = slice(k * 256 + pp * 128, k * 256 + pp * 128 + 128)
                        ksl = slice(k * CH, (k + 1) * CH)
                        nc.tensor.matmul(
                            pt[:, 0:512], T["wk2"][0:64, lsl],
                            xk_ch[0:64, ksl], start=(k == 0), stop=(k == 7),
                        )
                        nc.tensor.matmul(
                            pt[:, 512:1024], T["wk2"][64:128, lsl],
                            xk_ch[64:128, ksl], start=(k == 0), stop=(k == 7),
                        )
                    nc.vector.tensor_scalar_add(
                        T["KT"][pp][:, toks], pt[:, 0:512], bk_sb[:, m : m + 1]
                    )
                    nc.vector.tensor_add(
                        T["KT"][pp][:, toks], T["KT"][pp][:, toks], pt[:, 512:1024]
                    )


            def a_qv_chunk(ps, ch, T):
                """Q and V projections for chunk ch of pass ps."""
                toks = slice(ch * CH, (ch + 1) * CH)
                xq_ch = xqp.tile([128, 8 * CH], BF16, tag="xq", name=f"xq{ps}{ch}")
                nc.gpsimd.dma_start(
                    xq_ch[:].rearrange("p (k m) -> p k m", k=8),
                    xq_d[:, toks].rearrange("(k p) m -> p k m", p=128),
                )
                for pp in range(2):
                    m = 2 * ps + pp
                    pt = stp.tile([128, 1024], F32, tag="st", name=f"aq{ps}{ch}{pp}")
                    for k in range(8):
                        lsl = slice(k * 256 + pp * 128, k * 256 + pp * 128 + 128)
                        ksl = slice(k * CH, (k + 1) * CH)
                        nc.tensor.matmul(
                            pt[:, 0:512], T["wq2"][0:64, lsl],
                            xq_ch[0:64, ksl], start=(k == 0), stop=(k == 7),
                        )
                        nc.tensor.matmul(
                            pt[:, 512:1024], T["wq2"][64:128, lsl],
                            xq_ch[64:128, ksl], start=(k == 0), stop=(k == 7),
                        )
                    nc.vector.tensor_scalar_add(
                        T["QT"][pp][:, toks], pt[:, 0:512], bq_sb[:, m : m + 1]
                    )
                    nc.vector.tensor_add(
                        T["QT"][pp][:, toks], T["QT"][pp][:, toks], pt[:, 512:1024]
                    )

            def a_v_chunk(ch, wv_full, T0, T1):
                """V projection for BOTH passes (8 heads, N=512), chunk ch."""
                toks = slice(ch * CH, (ch + 1) * CH)
                xv_ch = xvp.tile([128, 8 * CH], BF16, tag="xv", name=f"xv{ch}")
                nc.gpsimd.dma_start(
                    xv_ch[:].rearrange("p (k m) -> p k m", k=8),
                    xv_d[:, toks].rearrange("(k p) m -> p k m", p=128),
                )
                for tt in range(CH // 128):
                    t = ch * (CH // 128) + tt
                    pt = stp.tile([128, 1024], F32, tag="st", name=f"av{ch}{tt}")
                    for k in range(8):
                        xsl = slice(k * CH + tt * 128, k * CH + tt * 128 + 128)
                        wsl = slice(k * 512, (k + 1) * 512)
                        nc.tensor.matmul(
                            pt[:, 0:512], xv_ch[0:64, xsl], wv_full[0:64, wsl],
                            start=(k == 0), stop=(k == 7),
                        )
                        nc.tensor.matmul(
                            pt[:, 512:1024], xv_ch[64:128, xsl],
                            wv_full[64:128, wsl], start=(k == 0), stop=(k == 7),
                        )
                    for ps, T in ((0, T0), (1, T1)):
                        dst = T["v"][:, t * 264 : (t + 1) * 264].rearrange(
                            "p (h c) -> p h c", h=4
                        )[:, :, 0:64]
                        lo = pt[:, ps * 256 : ps * 256 + 256].rearrange(
                            "p (h c) -> p h c", h=4
                        )
                        hi = pt[:, 512 + ps * 256 : 512 + ps * 256 + 256].rearrange(
                            "p (h c) -> p h c", h=4
                        )
                        bvb = bv_sb[:, ps * 256 : (ps + 1) * 256].rearrange(
                            "p (h c) -> p h c", h=4
                        )
                        nc.vector.tensor_add(dst, lo, bvb)
                        nc.vector.tensor_add(dst, dst, hi)

            def b_unit(ps, q, pp, T, cb=None):
                """Attention unit: pass ps, query block q (512 toks), pair pp."""
                qsl = slice(q * 512, (q + 1) * 512)
                oAB = op.tile([128, 1024], F32, tag="oAB", name=f"o{ps}{q}{pp}")
                for kt in range(NKT):
                    ksl = slice(kt * 128, (kt + 1) * 128)
                    st = stp.tile([128, 1024], F32, tag="st", name=f"st{ps}{q}{pp}{kt}")
                    nc.tensor.matmul(
                        st[:, 0:512], T["KT"][pp][0:64, ksl], T["QT"][pp][0:64, qsl],
                        start=True, stop=True,
                    )
                    nc.tensor.matmul(
                        st[:, 512:1024], T["KT"][pp][64:128, ksl],
                        T["QT"][pp][64:128, qsl], start=True, stop=True,
                    )
                    pt = ptp.tile([128, 1024], BF16, tag="pt", name=f"pt{ps}{q}{pp}{kt}")
                    nc.scalar.activation(pt[:], st[:], AF.Exp, scale=0.125)
                    base = kt * 264 + (2 * pp) * 66
                    nc.tensor.matmul(
                        oAB[0:66, 0:512], T["v"][:, base : base + 66], pt[:, 0:512],
                        start=(kt == 0), stop=(kt == NKT - 1),
                    )
                    nc.tensor.matmul(
                        oAB[0:66, 512:1024], T["v"][:, base + 66 : base + 132],
                        pt[:, 512:1024],
                        start=(kt == 0), stop=(kt == NKT - 1),
                    )
                    if cb is not None:
                        cb(kt)
                # drains: attn out to AT (bf16); den rows via DMA to su
                nc.vector.tensor_copy(T["AT"][pp][0:64, qsl], oAB[0:64, 0:512])
                nc.vector.tensor_copy(T["AT"][pp][64:128, qsl], oAB[0:64, 512:1024])
                su = smp.tile([128, 512], F32, tag="su", name=f"su{ps}{q}{pp}")
                su_r = smp.tile([128, 512], F32, tag="sur", name=f"sur{ps}{q}{pp}")
                nc.vector.memset(su[:], 1.0)
                nc.vector.tensor_copy(su[0:1, :], oAB[64:65, 0:512])
                nc.vector.tensor_copy(su[32:33, :], oAB[64:65, 512:1024])
                nc.vector.reciprocal(su_r[:], su[:])
                rc_u = drp.tile([2, 512], F32, tag="rc", name=f"rc{ps}{q}{pp}")
                nc.sync.dma_start(rc_u[0:1, :], su_r[0:1, :])
                nc.sync.dma_start(rc_u[1:2, :], su_r[32:33, :])
                rbc = rbp.tile([128, 512], F32, tag="rbc", name=f"rbc{ps}{q}{pp}")
                nc.sync.dma_start(rbc[0:64, :], rc_u[0:1, :].to_broadcast((64, 512)))
                nc.sync.dma_start(rbc[64:128, :], rc_u[1:2, :].to_broadcast((64, 512)))
                nc.vector.tensor_mul(T["AT"][pp][:, qsl], T["AT"][pp][:, qsl], rbc[:])

            def load_wo(ps, T):
                wo_sb = wp.tile([128, 2048], BF16, tag="wo", name=f"wo{ps}")
                for kb in range(2):
                    rs = slice(ps * 256 + kb * 128, ps * 256 + kb * 128 + 128)
                    nc.gpsimd.dma_start(wo_sb[:, kb * 1024 : (kb + 1) * 1024], wo_d[rs, :])
                T["wo"] = wo_sb

            def c_chunk(ps, mp, T):
                """Output projection for token tile mp of pass ps."""
                msl = slice(mp * 128, (mp + 1) * 128)
                oc = ocp.tile([128, 1024], F32, tag="oc", name=f"oc{ps}{mp}")
                for n in range(2):
                    ps_t = stp.tile([128, 1024], F32, tag="st", name=f"c{ps}{mp}{n}")
                    for kb in range(2):
                        wsl = slice(kb * 1024 + n * 512, kb * 1024 + n * 512 + 512)
                        nc.tensor.matmul(
                            ps_t[:, 0:512], T["AT"][kb][0:64, msl],
                            T["wo"][0:64, wsl], start=(kb == 0), stop=(kb == 1),
                        )
                        nc.tensor.matmul(
                            ps_t[:, 512:1024], T["AT"][kb][64:128, msl],
                            T["wo"][64:128, wsl], start=(kb == 0), stop=(kb == 1),
                        )
                    nc.vector.tensor_copy(
                        oc[:, n * 512 : (n + 1) * 512], ps_t[:, 0:512]
                    )
                    nc.vector.tensor_add(
                        oc[:, n * 512 : (n + 1) * 512],
                        oc[:, n * 512 : (n + 1) * 512], ps_t[:, 512:1024]
                    )
                nc.gpsimd.dma_start(out_d[ps][msl, :], oc[:])

            def sink(tiles):
                oc = ocp.tile([128, 1024], F32, tag="oc", name=f"sink{sink.n}")
                sink.n += 1
                for t in tiles:
                    nc.vector.tensor_copy(oc[:, 0:512], t[0:128, 0:512])
                nc.sync.dma_start(out_d[0][0:128, :], oc[:])
            sink.n = 0

            def stage_a(ps, T, wv_full, To):
                load_pass_weights(ps, T)
                memset_ones(T)
                for ch in range(NCH):
                    a_k_chunk(ps, ch, T)
                for ch in range(NCH):
                    a_qv_chunk(ps, ch, T)
                    a_v_chunk(ch, wv_full, T, To)

            def body():
                T0 = pass_tiles(0)
                T1 = pass_tiles(1)
                wv_full = load_wv_full()
                load_pass_weights(0, T0)
                memset_ones(T0)
                memset_ones(T1)
                if phases == "a":
                    for ch in range(NCH):
                        a_k_chunk(0, ch, T0)
                    for ch in range(NCH):
                        a_qv_chunk(0, ch, T0)
                        a_v_chunk(ch, wv_full, T0, T1)
                    load_pass_weights(1, T1)
                    for ch in range(NCH):
                        a_k_chunk(1, ch, T1)
                    for ch in range(NCH):
                        a_qv_chunk(1, ch, T1)
                    sink(T0["QT"] + T0["KT"] + [T0["v"]])
                    sink(T1["QT"] + T1["KT"] + [T1["v"]])
                    return
                do_c = "c" in phases
                if do_c:
                    load_wo(0, T0)
                    load_wo(1, T1)
                load_pass_weights(1, T1)
                # PE filler work queue: thunks issued 2 per B unit.
                # NOTE: Tile dependency tracking is trace-order-based, so
                # every tile READ must be traced after its write. A pass's
                # full KT must therefore be traced before its first b_unit.
                filler = []
                for ch in range(1, NCH):
                    filler.append(lambda ch=ch: a_qv_chunk(0, ch, T0))
                for ch in range(NCH):
                    filler.append(lambda ch=ch: a_k_chunk(1, ch, T1))
                    filler.append(lambda ch=ch: a_qv_chunk(1, ch, T1))
                # prologue: pair-0 K chunks + Q/V chunk 0 of pass 0.
                # pair-1 K is the first filler item, traced during unit 0
                # (unit 1 = (q0, pp1) is the first reader of KT[1]).
                for ch in range(NCH):
                    a_k_chunk(0, ch, T0, pps=(0,))
                a_qv_chunk(0, 0, T0)
                a_v_chunk(0, wv_full, T0, T1)

                def unit0_cb(kt):
                    # feed remaining V chunks while unit 0 streams kt tiles
                    if kt in (3, 7, 11):
                        a_v_chunk(kt // 4 + 1, wv_full, T0, T1)

                units = [(ps, q, pp) for ps in range(2) for q in range(NQ)
                         for pp in range(2)]
                c_ready = []
                for i, (ps, q, pp) in enumerate(units):
                    T = T0 if ps == 0 else T1
                    b_unit(ps, q, pp, T, cb=unit0_cb if i == 0 else None)
                    if i == 0:
                        # pair-1 K projections: traced after unit 0 (their
                        # first reader is unit 1), issued here so the
                        # prologue only pays for pair-0 K.
                        for ch in range(NCH):
                            a_k_chunk(0, ch, T0, pps=(1,))
                    if do_c and pp == 1:
                        c_ready.extend((ps, mp) for mp in range(4 * q, 4 * q + 4))
                    if i >= 1:
                        for _ in range(2):
                            if filler:
                                filler.pop(0)()
                    if do_c:
                        for _ in range(3 if i >= 12 else 2):
                            if c_ready:
                                cps, mp = c_ready.pop(0)
                                c_chunk(cps, mp, T0 if cps == 0 else T1)
                    elif i % 4 == 3:
                        sink([T["AT"][pp]])
                while filler:
                    filler.pop(0)()
                if do_c:
                    while c_ready:
                        cps, mp = c_ready.pop(0)
                        c_chunk(cps, mp, T0 if cps == 0 else T1)

            if reps == 0:
                body()
            else:
                with tc.For_i(
                    0, reps, 1,
                    hint_engines=(
                        mybir.EngineType.PE,
                        mybir.EngineType.Activation,
                        mybir.EngineType.DVE,
                        mybir.EngineType.SP,
                    ),
                ):
                    body()

    nc.compile()
    return nc


def make_in_maps(query, key, value, Wq, bq, Wk, bk, Wv, bv, Wo, bo):
    """Host-side sharding: per-core input dicts (8 cores), bf16."""
    import ml_dtypes

    def bf(x):
        return np.ascontiguousarray(np.asarray(x, np.float32)).astype(
            ml_dtypes.bfloat16
        )

    qT = [bf(np.asarray(query[b]).T) for b in range(4)]
    kT = [bf(np.asarray(key[b]).T) for b in range(4)]
    vT = [bf(np.asarray(value[b]).T) for b in range(4)]
    in_maps = []
    for c in range(8):
        b, g = c % 4, c // 4
        hs = slice(g * 512, (g + 1) * 512)
        in_maps.append(
            {
                "xq": qT[b],
                "xk": kT[b],
                "xv": vT[b],
                "wq": bf(np.asarray(Wq)[hs, :].T),
                "wk": bf(np.asarray(Wk)[hs, :].T),
                "wv": bf(np.asarray(Wv)[hs, :].T),
                "wo": bf(np.asarray(Wo)[:, hs].T),
                "bq": np.ascontiguousarray(
                    np.asarray(bq, np.float32)[hs].reshape(4, 128, 1)
                ),
                "bk": np.ascontiguousarray(
                    np.asarray(bk, np.float32)[hs].reshape(4, 128, 1)
                ),
                "bv": np.ascontiguousarray(
                    np.asarray(bv, np.float32)[hs].reshape(1, 512)
                ),
            }
        )
    return in_maps


def assemble(results, bo):
    """Sum partials: out[b] = sum over half g, pass p of core partials + bo."""
    out = np.zeros((4, S, D), np.float32)
    for c in range(8):
        b = c % 4
        out[b] += results[c]["out0"]
        out[b] += results[c]["out1"]
    out += np.asarray(bo, np.float32)[None, None, :]
    return out


_NC_CACHE = {}


def kernel(query, key, value, Wq, bq, Wk, bk, Wv, bv, Wo, bo, *, nc=None):
    in_maps = make_in_maps(query, key, value, Wq, bq, Wk, bk, Wv, bv, Wo, bo)
    if nc is None:
        if "nc" not in _NC_CACHE:
            _NC_CACHE["nc"] = build(reps=0)
        nc = _NC_CACHE["nc"]
    res = bass_utils.run_bass_kernel_spmd(nc, in_maps, core_ids=list(range(8)))
    return assemble(res.results, bo)



# revision 5
# speedup vs baseline: 1.1034x; 1.1034x over previous
"""Multi-head attention TRN2 kernel v4 (8-core SPMD, batch x head-half).

Per core (c): batch b = c % 4, head-half g = c // 4 (8 of 16 heads),
processed as 2 passes x 2 pairs x 4 q-blocks (512 toks) = 16 attention
units of 16 key tiles each. bf16 matmul dataflow as v2/v3:
  QT/KT [2x64 hd, tok] pair-major; V token-major with ones column per
  head (softmax denominator from the attn@V matmul, row 64).

v4: globally software-pipelined single stream. ACT exp is the scarce
resource (256 x [128,1024] activations ~ 1.15us = ~293us floor); the
stream traces, per (unit, kt) step t: exp(t) -> QK(t+1) -> AV(t), so PE
runs one score tile ahead of ACT and exp(t+1)'s input is always ready
when ACT frees up. All projections (Q/K both passes, V once for both)
and the output projection are decomposed into ~213ns PE micro-op pairs
fed into per-step PE slack by a deadline scheduler (forced when their
first reader is imminent, eager otherwise, ~2 pairs/step).
PSUM: st 2x[128,1024] + fil 1 + oAB 1 = 8 banks. xk is SBUF-resident
(32KB/part) so pass 1 reuses it; xq streams twice; xv streams once.
"""
import numpy as np
import concourse.bacc as bacc
import concourse.mybir as mybir
import concourse.tile as tile
from concourse import bass_utils

BF16 = mybir.dt.bfloat16
F32 = mybir.dt.float32
AF = mybir.ActivationFunctionType

S, D = 2048, 1024
CH = 512          # token chunk for projections
NCH = S // CH     # 4
NKT = S // 128    # 16 key-token tiles
NQ = S // 512     # 4 query blocks of 512
NSTEP = 16 * NKT  # 16 units x 16 kt steps
BIG = 1 << 30


def build(reps: int = 1, phases: str = "abc"):
    nc = bacc.Bacc("TRN2", target_bir_lowering=False, debug=False, num_devices=8)
    xq_d = nc.dram_tensor("xq", [D, S], BF16, kind="ExternalInput")
    xk_d = nc.dram_tensor("xk", [D, S], BF16, kind="ExternalInput")
    xv_d = nc.dram_tensor("xv", [D, S], BF16, kind="ExternalInput")
    wq_d = nc.dram_tensor("wq", [D, 512], BF16, kind="ExternalInput")
    wk_d = nc.dram_tensor("wk", [D, 512], BF16, kind="ExternalInput")
    wv_d = nc.dram_tensor("wv", [D, 512], BF16, kind="ExternalInput")
    wo_d = nc.dram_tensor("wo", [512, D], BF16, kind="ExternalInput")
    bq_d = nc.dram_tensor("bq", [4, 128, 1], F32, kind="ExternalInput")
    bk_d = nc.dram_tensor("bk", [4, 128, 1], F32, kind="ExternalInput")
    bv_d = nc.dram_tensor("bv", [1, 512], F32, kind="ExternalInput")
    out_d = [
        nc.dram_tensor(f"out{p}", [S, D], F32, kind="ExternalOutput") for p in (0, 1)
    ]

    units = [(ps, q, pp) for ps in range(2) for q in range(NQ) for pp in range(2)]

    with tile.TileContext(nc) as tc:
        with (
            tc.tile_pool(name="pers", bufs=1) as pers,
            tc.tile_pool(name="xqp", bufs=2) as xqp,
            tc.tile_pool(name="xvp", bufs=2) as xvp,
            tc.tile_pool(name="wp", bufs=2) as wp,
            tc.tile_pool(name="qkvp", bufs=2) as qkvp,
            tc.tile_pool(name="ptp", bufs=10) as ptp,
            tc.tile_pool(name="atp", bufs=2) as atp,
            tc.tile_pool(name="smp", bufs=2) as smp,
            tc.tile_pool(name="rbp", bufs=3) as rbp,
            tc.tile_pool(name="ocp", bufs=2) as ocp,
            tc.tile_pool(name="stp", bufs=2, space="PSUM") as stp,
            tc.tile_pool(name="op", bufs=1, space="PSUM") as op,
            tc.tile_pool(name="drp", bufs=3, space="DRAM") as drp,
        ):
            bq_sb = pers.tile([128, 4], F32, tag="bq")
            bk_sb = pers.tile([128, 4], F32, tag="bk")
            for m in range(4):
                nc.sync.dma_start(bq_sb[:, m : m + 1], bq_d[m])
                nc.sync.dma_start(bk_sb[:, m : m + 1], bk_d[m])
            bv_sb = pers.tile([128, 512], F32, tag="bv")
            nc.sync.dma_start(bv_sb[:], bv_d[:].to_broadcast((128, 512)))
            # dummy exp: pulls the one-time ACT Exp-table load into the
            # prologue so the first real softmax tile isn't delayed by it
            warm = pers.tile([128, 4], BF16, tag="warm")
            nc.scalar.activation(warm[:], bq_sb[:], AF.Exp, scale=0.0)
            # resident full xk [128, k=8 x tok] (both passes read it)
            xk_sb = pers.tile([128, 8 * S], BF16, tag="xk")

            def body():
                T = []
                for ps in range(2):
                    QT = [
                        qkvp.tile([128, S], BF16, tag=f"qt{pp}", name=f"QT{ps}{pp}")
                        for pp in range(2)
                    ]
                    KT = [
                        qkvp.tile([128, S], BF16, tag=f"kt{pp}", name=f"KT{ps}{pp}")
                        for pp in range(2)
                    ]
                    v_sb = qkvp.tile([128, NKT * 264], BF16, tag="v", name=f"v{ps}")
                    AT = [
                        atp.tile([128, S], BF16, tag=f"at{pp}", name=f"AT{ps}{pp}")
                        for pp in range(2)
                    ]
                    T.append(dict(QT=QT, KT=KT, v=v_sb, AT=AT))

                wq2, wk2, wo_t, xq_t, xv_t = {}, {}, {}, {}, {}

                def dma_wqk(ps):
                    cs = slice(ps * 256, (ps + 1) * 256)
                    wq2[ps] = wp.tile([128, 2048], BF16, tag="wq2", name=f"wq2{ps}")
                    wk2[ps] = wp.tile([128, 2048], BF16, tag="wk2", name=f"wk2{ps}")
                    for w_sb, w_d in ((wq2[ps], wq_d), (wk2[ps], wk_d)):
                        nc.gpsimd.dma_start(
                            w_sb[:].rearrange("p (k m) -> p k m", k=8),
                            w_d[:, cs].rearrange("(k p) m -> p k m", p=128),
                        )

                def dma_wo(ps):
                    wo_sb = wp.tile([128, 2048], BF16, tag="wo", name=f"wo{ps}")
                    for kb in range(2):
                        rs = slice(ps * 256 + kb * 128, ps * 256 + kb * 128 + 128)
                        nc.gpsimd.dma_start(
                            wo_sb[:, kb * 1024 : (kb + 1) * 1024], wo_d[rs, :]
                        )
                    wo_t[ps] = wo_sb

                def dma_xk(ch):
                    toks = slice(ch * CH, (ch + 1) * CH)
                    nc.gpsimd.dma_start(
                        xk_sb[:].rearrange("p (k m) -> p k m", k=8)[:, :, toks],
                        xk_d[:, toks].rearrange("(k p) m -> p k m", p=128),
                    )

                def dma_xq(ps, ch):
                    toks = slice(ch * CH, (ch + 1) * CH)
                    xq_ch = xqp.tile([128, 8 * CH], BF16, tag="xq", name=f"xq{ps}{ch}")
                    nc.gpsimd.dma_start(
                        xq_ch[:].rearrange("p (k m) -> p k m", k=8),
                        xq_d[:, toks].rearrange("(k p) m -> p k m", p=128),
                    )
                    xq_t[(ps, ch)] = xq_ch

                def dma_xv(ch):
                    toks = slice(ch * CH, (ch + 1) * CH)
                    xv_ch = xvp.tile([128, 8 * CH], BF16, tag="xv", name=f"xv{ch}")
                    nc.sync.dma_start(
                        xv_ch[:].rearrange("p (k m) -> p k m", k=8),
                        xv_d[:, toks].rearrange("(k p) m -> p k m", p=128),
                    )
                    xv_t[ch] = xv_ch

                def memset_ones(ps):
                    for t in range(NKT):
                        vv = T[ps]["v"][:, t * 264 : (t + 1) * 264].rearrange(
                            "p (h c) -> p h c", h=4
                        )
                        nc.vector.memset(vv[:, :, 64:66], 0.0)
                        nc.vector.memset(vv[:, :, 64:65], 1.0)

                # ---- micro-op chunk builders (lists of (cost, fn)) ----
                def kq_thunks(ps, ch, pp, which):
                    toks = slice(ch * CH, (ch + 1) * CH)
                    m = 2 * ps + pp
                    hold = {}

                    def mk(k):
                        def f():
                            if k == 0:
                                hold["t"] = stp.tile(
                                    [128, 1024], F32, tag="fil", bufs=1,
                                    name=f"f{which}{ps}{ch}{pp}",
                                )
                            pt = hold["t"]
                            w_sb = wk2[ps] if which == "k" else wq2[ps]
                            lsl = slice(
                                k * 256 + pp * 128, k * 256 + pp * 128 + 128
                            )
                            if which == "k":
                                xsl = slice(k * S + ch * CH, k * S + (ch + 1) * CH)
                                mlo, mhi = xk_sb[0:64, xsl], xk_sb[64:128, xsl]
                            else:
                                xq_ch = xq_t[(ps, ch)]
                                xsl = slice(k * CH, (k + 1) * CH)
                                mlo, mhi = xq_ch[0:64, xsl], xq_ch[64:128, xsl]
                            nc.tensor.matmul(
                                pt[:, 0:512], w_sb[0:64, lsl], mlo,
                                start=(k == 0), stop=(k == 7),
                            )
                            nc.tensor.matmul(
                                pt[:, 512:1024], w_sb[64:128, lsl], mhi,
                                start=(k == 0), stop=(k == 7),
                            )

                        return (1, f)

                    def fin():
                        dest = (T[ps]["KT"] if which == "k" else T[ps]["QT"])[pp]
                        b_sb = bk_sb if which == "k" else bq_sb
                        pt = hold["t"]
                        nc.vector.tensor_scalar_add(
                            dest[:, toks], pt[:, 0:512], b_sb[:, m : m + 1]
                        )
                        nc.vector.tensor_add(
                            dest[:, toks], dest[:, toks], pt[:, 512:1024]
                        )

                    return [mk(k) for k in range(8)] + [(0, fin)]

                def v_tt_thunks(ch, tt):
                    ti = ch * 4 + tt
                    hold = {}

                    def mk(k):
                        def f():
                            if k == 0:
                                hold["t"] = stp.tile(
                                    [128, 1024], F32, tag="fil", bufs=1,
                                    name=f"av{ti}",
                                )
                            pt = hold["t"]
                            xv_ch = xv_t[ch]
                            xsl = slice(k * CH + tt * 128, k * CH + tt * 128 + 128)
                            wsl = slice(k * 512, (k + 1) * 512)
                            nc.tensor.matmul(
                                pt[:, 0:512], xv_ch[0:64, xsl], wv_full[0:64, wsl],
                                start=(k == 0), stop=(k == 7),
                            )
                            nc.tensor.matmul(
                                pt[:, 512:1024], xv_ch[64:128, xsl],
                                wv_full[64:128, wsl], start=(k == 0), stop=(k == 7),
                            )

                        return (1, f)

                    def fin():
                        pt = hold["t"]
                        for ps in range(2):
                            dst = T[ps]["v"][
                                :, ti * 264 : (ti + 1) * 264
                            ].rearrange("p (h c) -> p h c", h=4)[:, :, 0:64]
                            lo = pt[:, ps * 256 : ps * 256 + 256].rearrange(
                                "p (h c) -> p h c", h=4
                            )
                            hi = pt[
                                :, 512 + ps * 256 : 512 + ps * 256 + 256
                            ].rearrange("p (h c) -> p h c", h=4)
                            bvb = bv_sb[:, ps * 256 : (ps + 1) * 256].rearrange(
                                "p (h c) -> p h c", h=4
                            )
                            nc.vector.tensor_add(dst, lo, bvb)
                            nc.vector.tensor_add(dst, dst, hi)

                    return [mk(k) for k in range(8)] + [(0, fin)]

                def c_thunks(ps, mp):
                    msl = slice(mp * 128, (mp + 1) * 128)
                    hold = {}

                    def mk(n, kb):
                        def f():
                            if kb == 0:
                                hold[n] = stp.tile(
                                    [128, 1024], F32, tag="fil", bufs=1,
                                    name=f"c{ps}{mp}{n}",
                                )
                                if n == 0:
                                    hold["oc"] = ocp.tile(
                                        [128, 1024], F32, tag="oc",
                                        name=f"oc{ps}{mp}",
                                    )
                            pt = hold[n]
                            wsl = slice(
                                kb * 1024 + n * 512, kb * 1024 + n * 512 + 512
                            )
                            nc.tensor.matmul(
                                pt[:, 0:512], T[ps]["AT"][kb][0:64, msl],
                                wo_t[ps][0:64, wsl],
                                start=(kb == 0), stop=(kb == 1),
                            )
                            nc.tensor.matmul(
                                pt[:, 512:1024], T[ps]["AT"][kb][64:128, msl],
                                wo_t[ps][64:128, wsl],
                                start=(kb == 0), stop=(kb == 1),
                            )

                        return (1, f)

                    def dve(n):
                        def f():
                            osl = hold["oc"][:, n * 512 : (n + 1) * 512]
                            nc.vector.tensor_copy(osl, hold[n][:, 0:512])
                            nc.vector.tensor_add(osl, osl, hold[n][:, 512:1024])

                        return (0, f)

                    def out():
                        nc.gpsimd.dma_start(out_d[ps][msl, :], hold["oc"][:])

                    return [mk(0, 0), mk(0, 1), dve(0), mk(1, 0), mk(1, 1),
                            dve(1), (0, out)]

                # ---- main-stream tracers ----
                st_tiles, pt_tiles, oab = {}, {}, {}

                def trace_qk(t2):
                    u2, kt2 = divmod(t2, NKT)
                    ps2, q2, pp2 = units[u2]
                    st_t = stp.tile([128, 1024], F32, tag="st", name=f"st{t2}")
                    ksl = slice(kt2 * 128, (kt2 + 1) * 128)
                    qsl = slice(q2 * 512, (q2 + 1) * 512)
                    KT, QT = T[ps2]["KT"][pp2], T[ps2]["QT"][pp2]
                    nc.tensor.matmul(
                        st_t[:, 0:512], KT[0:64, ksl], QT[0:64, qsl],
                        start=True, stop=True,
                    )
                    nc.tensor.matmul(
                        st_t[:, 512:1024], KT[64:128, ksl], QT[64:128, qsl],
                        start=True, stop=True,
                    )
                    st_tiles[t2] = st_t

                def trace_exp(t):
                    pt = ptp.tile([128, 1024], BF16, tag="pt", name=f"pt{t}")
                    nc.scalar.activation(
                        pt[:], st_tiles.pop(t)[:], AF.Exp, scale=0.125
                    )
                    pt_tiles[t] = pt

                def trace_av(t):
                    u, kt = divmod(t, NKT)
                    ps, q, pp = units[u]
                    if kt == 0:
                        oab[u] = op.tile([128, 1024], F32, tag="oAB", name=f"o{u}")
                    pt = pt_tiles.pop(t)
                    base = kt * 264 + (2 * pp) * 66
                    v_sb = T[ps]["v"]
                    nc.tensor.matmul(
                        oab[u][0:66, 0:512], v_sb[:, base : base + 66],
                        pt[:, 0:512], start=(kt == 0), stop=(kt == NKT - 1),
                    )
                    nc.tensor.matmul(
                        oab[u][0:66, 512:1024], v_sb[:, base + 66 : base + 132],
                        pt[:, 512:1024], start=(kt == 0), stop=(kt == NKT - 1),
                    )

                def drain(u):
                    ps, q, pp = units[u]
                    qsl = slice(q * 512, (q + 1) * 512)
                    oAB = oab.pop(u)
                    AT = T[ps]["AT"][pp]
                    nc.vector.tensor_copy(AT[0:64, qsl], oAB[0:64, 0:512])
                    nc.vector.tensor_copy(AT[64:128, qsl], oAB[0:64, 512:1024])
                    su = smp.tile([128, 512], F32, tag="su", name=f"su{u}")
                    su_r = smp.tile([128, 512], F32, tag="sur", name=f"sur{u}")
                    nc.vector.memset(su[:], 1.0)
                    nc.vector.tensor_copy(su[0:1, :], oAB[64:65, 0:512])
                    nc.vector.tensor_copy(su[32:33, :], oAB[64:65, 512:1024])
                    nc.vector.reciprocal(su_r[:], su[:])
                    rc_u = drp.tile([2, 512], F32, tag="rc", name=f"rc{u}")
                    nc.sync.dma_start(rc_u[0:1, :], su_r[0:1, :])
                    nc.sync.dma_start(rc_u[1:2, :], su_r[32:33, :])
                    rbc = rbp.tile([128, 512], F32, tag="rbc", name=f"rbc{u}")
                    nc.sync.dma_start(
                        rbc[0:64, :], rc_u[0:1, :].to_broadcast((64, 512))
                    )
                    nc.sync.dma_start(
                        rbc[64:128, :], rc_u[1:2, :].to_broadcast((64, 512))
                    )
                    nc.vector.tensor_mul(AT[:, qsl], AT[:, qsl], rbc[:])

                # ---- prologue ----
                wv_full = wp.tile([128, 4096], BF16, tag="wv", bufs=1, name="wvfull")
                nc.sync.dma_start(
                    wv_full[:].rearrange("p (k m) -> p k m", k=8),
                    wv_d[:, :].rearrange("(k p) m -> p k m", p=128),
                )
                dma_xv(0)
                dma_xk(0)
                dma_wqk(0)
                dma_xq(0, 0)
                for ch in range(1, NCH):
                    dma_xk(ch)
                memset_ones(0)
                memset_ones(1)
                for _, fn in kq_thunks(0, 0, 0, "k"):
                    fn()
                for _, fn in kq_thunks(0, 0, 0, "q"):
                    fn()
                for _, fn in v_tt_thunks(0, 0):
                    fn()

                # ---- filler queue: (deadline-of-last-thunk, thunks) ----
                raw = []
                for ch in range(NCH):
                    for tt in range(4):
                        ti = ch * 4 + tt
                        if ti == 0:
                            continue
                        raw.append((ti - 1, v_tt_thunks(ch, tt)))
                    if ch >= 1:
                        raw.append((4 * ch - 6, [(0, lambda ch=ch: dma_xv(ch))]))
                for ch in range(1, NCH):
                    raw.append((4 * ch - 2, kq_thunks(0, ch, 0, "k")))
                for ch in range(NCH):
                    raw.append((14 + 4 * ch, kq_thunks(0, ch, 1, "k")))
                raw.append((14, kq_thunks(0, 0, 1, "q")))
                for ch in range(1, NCH):
                    raw.append((32 * ch - 12, [(0, lambda ch=ch: dma_xq(0, ch))]))
                    for pp in range(2):
                        raw.append(((2 * ch + pp) * 16 - 2, kq_thunks(0, ch, pp, "q")))
                raw.append((40, [(0, lambda: dma_wo(0))]))
                raw.append((112, [(0, lambda: dma_wqk(1))]))
                raw.append((150, [(0, lambda: dma_wo(1))]))
                for ch in range(NCH):
                    raw.append((116 + 32 * ch, [(0, lambda ch=ch: dma_xq(1, ch))]))
                for ch in range(NCH):
                    for pp in range(2):
                        raw.append((126 + 16 * pp + 4 * ch, kq_thunks(1, ch, pp, "k")))
                        raw.append((126 + (2 * ch + pp) * 16, kq_thunks(1, ch, pp, "q")))
                raw.sort(key=lambda x: x[0])
                filq = []
                for dl_last, thunks in raw:
                    n = len(thunks)
                    filq.append(
                        [(dl_last - (n - 1 - i), c, f)
                         for i, (c, f) in enumerate(thunks)]
                    )

                cq = []
                pstate = dict(fi=0, cur=[])

                def pump(t, budget=2):
                    while True:
                        if not pstate["cur"]:
                            fi = pstate["fi"]
                            fil_ok = fi < len(filq)
                            fil_urgent = fil_ok and filq[fi][0][0] <= t + 8
                            take_fil = fil_ok and (fil_urgent or not cq)
                            if take_fil and (
                                budget > 0 or filq[fi][0][0] <= t
                            ):
                                pstate["cur"] = list(filq[fi])
                                pstate["fi"] += 1
                            elif cq and budget > 0:
                                pstate["cur"] = [
                                    (BIG, c, f) for c, f in cq.pop(0)
                                ]
                            else:
                                return budget
                        cur = pstate["cur"]
                        while cur and (budget > 0 or cur[0][0] <= t):
                            _, c, fn = cur.pop(0)
                            fn()
                            budget -= c
                        if cur:
                            return budget

                # ---- main pipelined stream ----
                trace_qk(0)
                for t in range(NSTEP):
                    u, kt = divmod(t, NKT)
                    trace_exp(t)
                    if t + 1 < NSTEP:
                        trace_qk(t + 1)
                    trace_av(t)
                    if kt == NKT - 1:
                        drain(u)
                        ps, q, pp = units[u]
                        if pp == 1 and "c" in phases:
                            for mp in range(4 * q, 4 * q + 4):
                                cq.append(c_thunks(ps, mp))
                    pump(t)
                # tail: flush all remaining fillers and c chunks
                while (
                    pstate["cur"] or pstate["fi"] < len(filq) or cq
                ):
                    pump(BIG, budget=BIG)

            if reps == 0:
                body()
            else:
                with tc.For_i(
                    0, reps, 1,
                    hint_engines=(
                        mybir.EngineType.PE,
                        mybir.EngineType.Activation,
                        mybir.EngineType.DVE,
                        mybir.EngineType.SP,
                    ),
                ):
                    body()

    nc.compile()
    return nc


def make_in_maps(query, key, value, Wq, bq, Wk, bk, Wv, bv, Wo, bo):
    """Host-side sharding: per-core input dicts (8 cores), bf16."""
    import ml_dtypes

    def bf(x):
        return np.ascontiguousarray(np.asarray(x, np.float32)).astype(
            ml_dtypes.bfloat16
        )

    qT = [bf(np.asarray(query[b]).T) for b in range(4)]
    kT = [bf(np.asarray(key[b]).T) for b in range(4)]
    vT = [bf(np.asarray(value[b]).T) for b in range(4)]
    in_maps = []
    for c in range(8):
        b, g = c % 4, c // 4
        hs = slice(g * 512, (g + 1) * 512)
        in_maps.append(
            {
                "xq": qT[b],
                "xk": kT[b],
                "xv": vT[b],
                "wq": bf(np.asarray(Wq)[hs, :].T),
                "wk": bf(np.asarray(Wk)[hs, :].T),
                "wv": bf(np.asarray(Wv)[hs, :].T),
                "wo": bf(np.asarray(Wo)[:, hs].T),
                "bq": np.ascontiguousarray(
                    np.asarray(bq, np.float32)[hs].reshape(4, 128, 1)
                ),
                "bk": np.ascontiguousarray(
                    np.asarray(bk, np.float32)[hs].reshape(4, 128, 1)
                ),
                "bv": np.ascontiguousarray(
                    np.asarray(bv, np.float32)[hs].reshape(1, 512)
                ),
            }
        )
    return in_maps


def assemble(results, bo):
    """Sum partials: out[b] = sum over half g, pass p of core partials + bo."""
    out = np.zeros((4, S, D), np.float32)
    for c in range(8):
        b = c % 4
        out[b] += results[c]["out0"]
        out[b] += results[c]["out1"]
    out += np.asarray(bo, np.float32)[None, None, :]
    return out


_NC_CACHE = {}


def kernel(query, key, value, Wq, bq, Wk, bk, Wv, bv, Wo, bo, *, nc=None):
    in_maps = make_in_maps(query, key, value, Wq, bq, Wk, bk, Wv, bv, Wo, bo)
    if nc is None:
        if "nc" not in _NC_CACHE:
            _NC_CACHE["nc"] = build(reps=0)
        nc = _NC_CACHE["nc"]
    res = bass_utils.run_bass_kernel_spmd(nc, in_maps, core_ids=list(range(8)))
    return assemble(res.results, bo)
